# revision 5
# baseline (speedup 1.0000x reference)
"""DifferentiableRoIAlignRotated on 8 TRN2 NeuronCores.

Strategy (pure data parallelism over ROIs, features replicated):
 - Host computes, in exact float32 reference arithmetic, the sampling
   row-pair indices and per-slot bilinear weights for every (roi, point).
 - Each core gathers 2 row-pairs (x0,x0+1 contiguous, 512 f32) per sample
   point from the HWC-layout feature map in DRAM via SWDGE dma_gather,
   applies the per-(point,row) slot weights with DVE tensor_scalar
   (per-partition scalars), and sums the 4 corners with a fixed 0/1
   stationary matrix on the tensor engine (PSUM accumulate).
 - Output is written per-core as [points, C] (1 KiB contiguous rows, full
   DMA rate); the host transposes to [K, C, 7, 7] while unsharding.
"""
import sys

for _p in ("/opt/trn_rl_repo", "/root/.axon_site/_ro/trn_rl_repo"):
    if _p not in sys.path:
        sys.path.append(_p)

import numpy as np
from concourse import tile, mybir
from concourse.ap import AP
from concourse.bass_utils import run_bass_kernel_spmd
from concourse.bacc import Bacc

# problem constants (hardcoded per spec)
N, C, H, W = 2, 256, 128, 128
K = 4096
OUT_H = OUT_W = 7
P = OUT_H * OUT_W          # 49 sample points per roi
SPATIAL_SCALE = 0.0625
N_CORES = 8
K_PER = K // N_CORES       # 512 rois per core
PTS = K_PER * P            # 25088 points per core
NJ = PTS * 2               # 50176 gathered row-pairs per core
JT = NJ // 128             # 392 j-tiles of 128 (= 64 points each)
# SWDGE descriptor-ring capacity caps one dma_gather at ~1024 indices
# (1536 wedges the NRT exec unit); 50176 = 49 * 1024 exactly.
CALLS = 49
IDX_PER_CALL = NJ // CALLS  # 1024
SLOTS = IDX_PER_CALL // 128  # 8 j-tiles per gather call
ROWS = N * H * W           # 32768 feature rows in (b, y, x) order

f32 = mybir.dt.float32
i16 = mybir.dt.int16

_CACHED_NC = None
LAST_RESULTS = None


def _host_precompute(rois):
    """Exact float32 mirror of the reference coordinate math.

    Returns (idx_flat, wsl_flat): per-point row-pair base indices (2 per
    point) into the flat (b*H*W) feature rows, and the 2 slot weights per
    row (x-base and x-base+1 columns) with clipping and zero-padding masks
    folded in.
    """
    rois = rois.astype(np.float32, copy=False)
    batch = rois[:, 0].astype(np.int32)

    # Coordinate math on jax-CPU in float32, op-for-op identical to the
    # reference, so sampling weights match its trig bit-for-bit.
    import jax
    import jax.numpy as jnp

    cpu = jax.devices("cpu")[0]
    with jax.default_device(cpu):
        r = jnp.asarray(rois)
        rf = r[:, 1:] * SPATIAL_SCALE
        cx, cy, w, h, theta = rf[:, 0], rf[:, 1], rf[:, 2], rf[:, 3], rf[:, 4]
        ys = jnp.linspace(-0.5, 0.5, OUT_H, dtype=r.dtype)
        xs = jnp.linspace(-0.5, 0.5, OUT_W, dtype=r.dtype)
        _y, _x = jnp.meshgrid(ys, xs, indexing="ij")
        bgx = _x.reshape(1, -1)
        bgy = _y.reshape(1, -1)
        cos_t = jnp.cos(theta)[:, None]
        sin_t = jnp.sin(theta)[:, None]
        gx = bgx * w[:, None]
        gy = bgy * h[:, None]
        x_sample = gx * cos_t - gy * sin_t + cx[:, None]
        y_sample = gx * sin_t + gy * cos_t + cy[:, None]
        x_grid = 2.0 * x_sample / max(W - 1, 1) - 1.0
        y_grid = 2.0 * y_sample / max(H - 1, 1) - 1.0
        ix = np.asarray(((x_grid + 1.0) * W - 1.0) * 0.5)   # (K, P)
        iy = np.asarray(((y_grid + 1.0) * H - 1.0) * 0.5)

    x0 = np.floor(ix)
    y0 = np.floor(iy)
    wx1 = ix - x0
    wy1 = iy - y0
    wx0 = np.float32(1.0) - wx1
    wy0 = np.float32(1.0) - wy1

    # per-x-corner validity and slot mapping onto the clipped pair base
    vx = [
        ((x0 >= 0) & (x0 <= W - 1)).astype(np.float32),
        ((x0 + 1 >= 0) & (x0 + 1 <= W - 1)).astype(np.float32),
    ]
    vy = [
        ((y0 >= 0) & (y0 <= H - 1)).astype(np.float32),
        ((y0 + 1 >= 0) & (y0 + 1 <= H - 1)).astype(np.float32),
    ]
    xb = np.clip(x0, 0, W - 2)                      # pair base column
    xslot = [np.clip(x0, 0, W - 1) - xb, np.clip(x0 + 1, 0, W - 1) - xb]
    yrow = [
        np.clip(y0, 0, H - 1).astype(np.int32),
        np.clip(y0 + 1, 0, H - 1).astype(np.int32),
    ]
    wxc = [wx0 * vx[0], wx1 * vx[1]]
    wyr = [wy0 * vy[0], wy1 * vy[1]]

    # row-pair flat indices, (K, P, 2)
    idx = np.stack(
        [batch[:, None] * (H * W) + yrow[r] * W + xb.astype(np.int32) for r in range(2)],
        axis=-1,
    ).astype(np.int16)

    # slot weights (K, P, 2 rows, 2 slots)
    wsl = np.zeros((K, P, 2, 2), np.float32)
    for r in range(2):
        for s in range(2):
            wsl[:, :, r, s] = wyr[r] * (
                (xslot[0] == s).astype(np.float32) * wxc[0]
                + (xslot[1] == s).astype(np.float32) * wxc[1]
            )
    return idx, wsl


PAIRS = JT // 2            # 196 psum pairs of 128 points
OGROUP = 14                # psum pairs per output DMA
OGROUPS = PAIRS // OGROUP  # 14
N_Q = 4                    # SWDGE queues for gather gen/drain overlap


def _build_nc():
    nc = Bacc("TRN2", target_bir_lowering=True, num_swdge_queues=N_Q)
    ft = nc.dram_tensor("ft", [ROWS, C], f32, kind="ExternalInput")
    idxs = nc.dram_tensor("idxs", [128, NJ // 16], i16, kind="ExternalInput")
    wts = nc.dram_tensor("wts", [128, JT, 2], f32, kind="ExternalInput")
    smat = nc.dram_tensor("smat", [128, 64], f32, kind="ExternalInput")
    # device output layout: [partition p, pair, c] with point = pair*128 + p;
    # per-partition-contiguous so output DMA descriptors are large
    out = nc.dram_tensor("out", [128, PAIRS, C], f32, kind="ExternalOutput")

    # overlapping row-pair view: row i -> 512 contiguous floats starting at
    # flat element i*C (pixels (i) and (i+1)); max base is ROWS-2.
    ft_pairs = AP(ft[:, :].tensor, 0, [[C, ROWS - 1], [1, 2 * C]])

    with tile.TileContext(nc) as tc:
        with (
            tc.tile_pool(name="const", bufs=1) as constp,
            tc.tile_pool(name="g", bufs=4) as gp,
            tc.tile_pool(name="v", bufs=4) as vp,
            tc.tile_pool(name="ps", bufs=4, space="PSUM") as psp,
            tc.tile_pool(name="o", bufs=2) as op,
        ):
            t_idx = constp.tile([128, NJ // 16], i16)
            nc.sync.dma_start(t_idx[:], idxs[:, :])
            t_w = constp.tile([128, JT, 2], f32)
            nc.sync.dma_start(t_w[:], wts[:, :, :])
            t_s = constp.tile([128, 64], f32)
            nc.sync.dma_start(t_s[:], smat[:, :])

            ncols = IDX_PER_CALL // 16  # idx columns per gather call
            stage = None
            for call in range(CALLS):
                gbuf = gp.tile([128, SLOTS, 2 * C], f32, tag="gbuf")
                nc.gpsimd.dma_gather(
                    gbuf[:, :, :],
                    ft_pairs,
                    t_idx[:, call * ncols:(call + 1) * ncols],
                    IDX_PER_CALL,
                    IDX_PER_CALL,
                    2 * C,
                    elem_step=C,
                    queue_num=call % N_Q,
                )
                for s in range(SLOTS):
                    t = call * SLOTS + s   # global j-tile = 64 points
                    vw = vp.tile([128, 2 * C], f32, tag="vw")
                    nc.vector.tensor_scalar_mul(
                        vw[:, 0:C], gbuf[:, s, 0:C], t_w[:, t, 0:1])
                    nc.vector.tensor_scalar_mul(
                        vw[:, C:2 * C], gbuf[:, s, C:2 * C], t_w[:, t, 1:2])
                    half = (t % 2) * 64
                    if t % 2 == 0:
                        psum = psp.tile([128, C], f32, tag="psum")
                    nc.tensor.matmul(psum[half:half + 64, :], t_s[:, :],
                                     vw[:, 0:C], start=True, stop=False)
                    nc.tensor.matmul(psum[half:half + 64, :], t_s[:, :],
                                     vw[:, C:2 * C], start=False, stop=True)
                    if t % 2 == 1:
                        pair = t // 2
                        if pair % OGROUP == 0:
                            stage = op.tile([128, OGROUP, C], f32, tag="stage")
                        nc.scalar.copy(stage[:, pair % OGROUP, :], psum[:, :])
                        if pair % OGROUP == OGROUP - 1:
                            g0 = (pair // OGROUP) * OGROUP
                            nc.sync.dma_start(out[:, g0:g0 + OGROUP, :],
                                              stage[:, :, :])
    nc.compile()
    return nc


def kernel(features, rois):
    global _CACHED_NC, LAST_RESULTS
    features = np.asarray(features, dtype=np.float32)
    rois = np.asarray(rois, dtype=np.float32)
    assert features.shape == (N, C, H, W) and rois.shape == (K, 6)

    # (b, y, x, c) flat rows
    ft = np.ascontiguousarray(features.transpose(0, 2, 3, 1).reshape(ROWS, C))

    idx, wsl = _host_precompute(rois)           # (K,P,2) i16, (K,P,2,2) f32

    # fixed 0/1 corner-sum matrix: psum[p, c] = sum_j S[j, p] * vw[j, c]
    S = np.zeros((128, 64), np.float32)
    S[np.arange(128), np.arange(128) // 2] = 1.0

    in_maps = []
    for core in range(N_CORES):
        k0 = core * K_PER
        idx_c = idx[k0:k0 + K_PER].reshape(NJ)          # j order: (pt, row)
        wsl_c = wsl[k0:k0 + K_PER].reshape(NJ, 2)
        idx_wrapped = np.tile(idx_c.reshape(NJ // 16, 16).T, (8, 1))
        wts_c = np.ascontiguousarray(
            wsl_c.reshape(JT, 128, 2).transpose(1, 0, 2))
        in_maps.append({
            "ft": ft,
            "idxs": np.ascontiguousarray(idx_wrapped),
            "wts": wts_c,
            "smat": S,
        })

    if _CACHED_NC is None:
        _CACHED_NC = _build_nc()
    res = run_bass_kernel_spmd(_CACHED_NC, in_maps, core_ids=list(range(N_CORES)))
    LAST_RESULTS = res

    out = np.empty((K, C, P), np.float32)
    for core in range(N_CORES):
        k0 = core * K_PER
        # device layout [p, pair, c] -> point-major [pts, c]
        o = res.results[core]["out"].transpose(1, 0, 2).reshape(PTS, C)
        out[k0:k0 + K_PER] = o.reshape(K_PER, P, C).transpose(0, 2, 1)
    return out.reshape(K, C, OUT_H, OUT_W)
